# revision 66
# baseline (speedup 1.0000x reference)
"""Trainium2 Bass kernel for nn_BertSelfAttention_7962869367489.

Dual-branch (self + cross/"knowledge") BERT attention, B=4, S=1024, K=512,
H=1024, NH=16, HD=64, fp32.

Sharding: 8 cores = (batch b in 0..3) x (head-group hg in 0..1, 8 heads each).
All six projections are column-split by head-group; per-head attention is
entirely core-local; output columns are disjoint per core, so the gather is a
pure concatenation (no collectives).

Per-core pipeline (bf16 operands everywhere on the PE; f32 PSUM accumulation):
  - hs/ehs arrive pre-transposed and pre-cast to bf16 from the host
    (hsT [H,S], ehsT [H,K]), weights pre-cast to bf16, so no on-device
    transposes or casts are needed and input DMA bytes are halved.
  - Projections: QT/KT/KQT/KKT = W.T @ srcT in transposed orientation (bf16
    outs); Vaug/KVaug = srcT.T @ Wv in normal orientation with an augmented
    column of 2.0, so the ctx matmul also produces 2*softmax-denominator,
    folding the (ctx+kctx)*0.5 branch average into the normalization.
  - Per head h: scoresT[t,s] = K_h @ Q_h^T (contraction HD=64); exp on ACT
    with per-partition mask bias and 1/8 scale, written bf16; after the
    branch's exps, ctx[s,d|den] accumulates in PSUM in NORMAL orientation
    via lhsT = e-chunk [t,128s], rhs = Vaug_h [t,65] -- 65-row bf16 matmuls,
    ~2.4x fewer PE rows than the transposed form and no output transposes.
    Accumulation groups run sc-outer/kt-inner so each PSUM region hosts one
    group at a time (interleaved groups in one bank clobber each other).
  - Normalization + branch merge on DVE straight out of PSUM; output DMA'd
    in four head-pair quarters.
  - Remaining projections are split into ~1.7us (jt, sc) units and drained
    between attention branches so PE projection work fills the ACT-bound
    exp windows; knowledge branch h runs right after self branch h.
"""
import numpy as np
import ml_dtypes
from contextlib import ExitStack

import concourse.bacc as bacc
import concourse.tile as tile
import concourse.mybir as mybir
from concourse.bass_utils import run_bass_kernel_spmd

F32 = mybir.dt.float32
BF16 = mybir.dt.bfloat16
AF = mybir.ActivationFunctionType
ALU = mybir.AluOpType

P = 128
S = 1024        # query length
TKS = 1024      # self-branch key length
TKK = 512       # knowledge-branch key length
H = 1024        # model dim (projection contraction)
HG = 512        # per-core output width (8 heads x 64)
NHL = 8         # heads per core
HD = 64
HC = H // P     # 8 contraction chunks
INV = 0.125     # 1/sqrt(64)

_CACHE = {}
_DUMP = False


def _build():
    nc = bacc.Bacc(target_bir_lowering=False, debug=False)

    hsT = nc.dram_tensor("hsT", [H, S], BF16, kind="ExternalInput")
    ehsT = nc.dram_tensor("ehsT", [H, TKK], BF16, kind="ExternalInput")
    w_in = {}
    b_in = {}
    for nm in ["q", "k", "v", "kq", "kk", "kv"]:
        w_in[nm] = nc.dram_tensor(f"w{nm}", [H, HG], BF16, kind="ExternalInput")
        b_in[nm] = nc.dram_tensor(f"b{nm}", [HG], F32, kind="ExternalInput")
    mask = nc.dram_tensor("mask", [TKS], F32, kind="ExternalInput")
    emask = nc.dram_tensor("emask", [TKK], F32, kind="ExternalInput")
    out = nc.dram_tensor("out", [S, HG], F32, kind="ExternalOutput")

    with tile.TileContext(nc) as tc, ExitStack() as ctx:
        const = ctx.enter_context(tc.tile_pool(name="const", bufs=1))
        persist = ctx.enter_context(tc.tile_pool(name="persist", bufs=1))
        epool = ctx.enter_context(tc.tile_pool(name="epool", bufs=17))
        smallp = ctx.enter_context(tc.tile_pool(name="smallp", bufs=2))
        snpool = ctx.enter_context(tc.tile_pool(name="snpool", bufs=3))
        psproj = ctx.enter_context(tc.tile_pool(name="psproj", bufs=2, space="PSUM"))
        psbig = ctx.enter_context(tc.tile_pool(name="psbig", bufs=2, space="PSUM"))
        psctx = ctx.enter_context(tc.tile_pool(name="psctx", bufs=2, space="PSUM"))

        # ---- constants (gpsimd/SWDGE queue, but the DMA engines are shared,
        # so these are emitted interleaved with the big loads below in
        # need-order to keep the startup-critical stream dense) ----
        mask_sb = const.tile([P, TKS // P], F32)
        emask_sb = const.tile([P, TKK // P], F32)
        bias_col = {}
        for nm in ["q", "k", "kq", "kk"]:
            bias_col[nm] = const.tile([P, 4], F32, name=f"bias_{nm}")
        bias_row = {}
        for nm in ["v", "kv"]:
            bias_row[nm] = const.tile([P, HG], F32, name=f"brow_{nm}")
        twos = const.tile([P, 1], F32)
        nc.vector.memset(twos, 2.0)

        def load_consts_early():
            for nm in ["q", "k"]:
                nc.sync.dma_start(
                    out=bias_col[nm],
                    in_=b_in[nm].ap().rearrange("(jt p) -> p jt", p=P))
            nc.sync.dma_start(out=mask_sb,
                              in_=mask.ap().rearrange("(kt p) -> p kt", p=P))

        def load_consts_late():
            nc.sync.dma_start(out=emask_sb,
                              in_=emask.ap().rearrange("(kt p) -> p kt", p=P))
            for nm in ["kq", "kk"]:
                nc.sync.dma_start(
                    out=bias_col[nm],
                    in_=b_in[nm].ap().rearrange("(jt p) -> p jt", p=P))

        # ---- persistent activations ----
        QT = persist.tile([P, 4, S], BF16)       # [j%128, jt, s]
        KT = persist.tile([P, 4, TKS], BF16)
        KQT = persist.tile([P, 4, S], BF16)
        KKT = persist.tile([P, 4, TKK], BF16)
        Vaug = persist.tile([P, TKS // P, NHL, HD + 1], BF16)   # [t%128, tt, h, d|2]
        KVaug = persist.tile([P, TKK // P, NHL, HD + 1], BF16)
        hsT_sb = persist.tile([P, HC, S], BF16)   # [h%128, hc, s]
        ehsT_sb = persist.tile([P, HC, TKK], BF16)
        wsb = {}
        for nm in ["q", "k", "v", "kq", "kk", "kv"]:
            wsb[nm] = persist.tile([P, HC, HG], BF16, name=f"w_{nm}")
        # output staging in two head-halves; DMA'd in four head-pair quarters
        out_half = [persist.tile([P, S // P, 4, HD], F32, name=f"out_half{i}",
                                 tag=f"out_half{i}") for i in range(2)]

        # ---- input DMAs (sync/HWDGE queue), ordered so the prelude's
        # dependencies (hsT, wq, wk) land first ----
        def load_rows(dst, src, half, rows, cols):
            nc.sync.dma_start(
                out=dst[:, half * (rows // 2):(half + 1) * (rows // 2), :],
                in_=src[half * (rows * P // 2):(half + 1) * (rows * P // 2), :]
                .rearrange("(hc p) s -> p hc s", p=P))

        def load_w(nm, hc0, hcn):
            nc.sync.dma_start(
                out=wsb[nm][:, hc0:hc0 + hcn, :],
                in_=w_in[nm][hc0 * P:(hc0 + hcn) * P, :].rearrange(
                    "(hc p) j -> p hc j", p=P))

        def load_w_jt0(nm, hc0, hcn):
            # jt0 column block only: 4x fewer startup-critical bytes (the
            # 256B-run descriptor penalty still nets a 2x faster arrival)
            nc.sync.dma_start(
                out=wsb[nm][:, hc0:hc0 + hcn, 0:P],
                in_=w_in[nm][hc0 * P:(hc0 + hcn) * P, 0:P].rearrange(
                    "(hc p) j -> p hc j", p=P))

        # startup-critical loads (prelude needs wq/wk jt0 + all of hsT),
        # split fine and interleaved so the first projection matmuls start
        # (and the PE p-state ramps) as early as possible
        load_w_jt0("q", 0, 2)
        nc.sync.dma_start(out=hsT_sb[:, 0:1, :], in_=hsT[0:P, :].rearrange(
            "(hc p) s -> p hc s", p=P))
        load_w_jt0("q", 2, 2)
        nc.sync.dma_start(out=hsT_sb[:, 1:2, :], in_=hsT[P:2 * P, :].rearrange(
            "(hc p) s -> p hc s", p=P))
        load_w_jt0("k", 0, 4)
        load_rows(hsT_sb, hsT, 1, HC // 2, S)   # hc 2-3
        load_w_jt0("q", 4, 4)
        load_rows(hsT_sb, hsT, 2, HC // 2, S)   # hc 4-5
        load_w_jt0("k", 4, 4)
        load_rows(hsT_sb, hsT, 3, HC // 2, S)   # hc 6-7
        load_consts_early()
        load_w("v", 0, HC)
        nc.sync.dma_start(out=bias_row["v"],
                           in_=b_in["v"].ap().unsqueeze(0).broadcast_to([P, HG]))
        load_w("kq", 0, HC)
        load_w("kk", 0, HC)
        load_rows(ehsT_sb, ehsT, 0, HC, TKK)
        load_rows(ehsT_sb, ehsT, 1, HC, TKK)
        load_w("kv", 0, HC)
        load_consts_late()
        # the remaining wq/wk column blocks (jt1-3)
        def load_w_jt13(nm):
            nc.sync.dma_start(
                out=wsb[nm][:, :, P:4 * P],
                in_=w_in[nm][:, P:4 * P].rearrange("(hc p) j -> p hc j", p=P))

        load_w_jt13("q")
        load_w_jt13("k")
        nc.sync.dma_start(out=bias_row["kv"],
                           in_=b_in["kv"].ap().unsqueeze(0).broadcast_to([P, HG]))

        # ---- projection emitters ----
        def proj_t_unit(nm, dst, srcT, jt, sc, nsc):
            """One (jt, sc) unit: 8 hc-chunk matmuls + bias-add."""
            w = 512 if nsc > 1 else TKK
            ps = psproj.tile([P, w], F32, name="psj", tag="psj")
            for hc in range(HC):
                nc.tensor.matmul(
                    ps, lhsT=wsb[nm][:, hc, jt * P:(jt + 1) * P],
                    rhs=srcT[:, hc, sc * w:(sc + 1) * w],
                    start=(hc == 0), stop=(hc == HC - 1))
            nc.vector.tensor_scalar_add(
                dst[:, jt, sc * w:(sc + 1) * w], ps,
                bias_col[nm][:, jt:jt + 1])

        def proj_v_part(nm, dst, srcT, tt, h0, nh):
            # V-projection for a head subrange: each attention branch reads
            # only its own head's Vaug column, so the head-6/7 parts carry
            # end-of-schedule deadlines and can fill the tail gaps
            ps = psproj.tile([P, nh * HD], F32, name=f"psv{tt}_{h0}",
                             tag="psj")
            for hc in range(HC):
                nc.tensor.matmul(
                    ps, lhsT=srcT[:, hc, tt * P:(tt + 1) * P],
                    rhs=wsb[nm][:, hc, h0 * HD:(h0 + nh) * HD],
                    start=(hc == 0), stop=(hc == HC - 1))
            nc.vector.scalar_tensor_tensor(
                out=dst[:, tt, h0:h0 + nh, 0:HD],
                in0=ps.rearrange("p (h d) -> p h d", h=nh),
                scalar=1.0,
                in1=bias_row[nm].rearrange(
                    "p (h d) -> p h d", h=NHL)[:, h0:h0 + nh, :],
                op0=ALU.mult, op1=ALU.add)
            nc.vector.tensor_copy(
                dst[:, tt, h0:h0 + nh, HD:HD + 1],
                twos.unsqueeze(1).broadcast_to([P, nh, 1]))

        # ---- prelude: Q/K jt0 with all four (proj, sc) accumulation groups
        # concurrent (two in the idle scores-psum slots), matmuls woven in
        # DMA-arrival order so the PE starts and p-state-ramps early ----
        pre_ps = {
            ("q", 0): psproj.tile([P, 512], F32, name="pre_q0", tag="psj"),
            ("k", 0): psproj.tile([P, 512], F32, name="pre_k0", tag="psj"),
            ("q", 1): psctx.tile([P, 512], F32, name="pre_q1", tag="ctxps"),
            ("k", 1): psctx.tile([P, 512], F32, name="pre_k1", tag="ctxps"),
        }
        for hc2 in range(HC // 2):
            for nm in ["q", "k"]:
                for hc in (2 * hc2, 2 * hc2 + 1):
                    for sc in range(2):
                        nc.tensor.matmul(
                            pre_ps[(nm, sc)],
                            lhsT=wsb[nm][:, hc, 0:P],
                            rhs=hsT_sb[:, hc, sc * 512:(sc + 1) * 512],
                            start=(hc == 0), stop=(hc == HC - 1))
        # bias-adds ordered so the first scores matmul's inputs (QT s-half 0
        # and KT key-half 0) complete first
        for nm, sc in [("q", 0), ("k", 0), ("q", 1), ("k", 1)]:
            dst = QT if nm == "q" else KT
            nc.vector.tensor_scalar_add(
                dst[:, 0, sc * 512:(sc + 1) * 512], pre_ps[(nm, sc)],
                bias_col[nm][:, 0:1])

        # ---- filler units: remaining projections, drained between branches
        def fill_unit(u):
            nm, jt, sc = u[0], u[1], u[2]
            if nm == "v":
                proj_v_part("v", Vaug, hsT_sb, u[1], u[2], u[3])
            elif nm == "kv":
                proj_v_part("kv", KVaug, ehsT_sb, u[1], u[2], u[3])
            elif nm == "kk":
                proj_t_unit("kk", KKT, ehsT_sb, jt, 0, 1)
            elif nm == "q":
                proj_t_unit("q", QT, hsT_sb, jt, sc, 2)
            elif nm == "k":
                proj_t_unit("k", KT, hsT_sb, jt, sc, 2)
            elif nm == "kq":
                proj_t_unit("kq", KQT, hsT_sb, jt, sc, 2)

        # gap -> units (after self(h) -> G(2h), after knl(h) -> G(2h+1)).
        # Units are woven INSIDE the following branch's kt loop (paced across
        # the kt steps) so ready filler work sits between the ACT-dependent
        # scores/ctx matmuls in the PE queue; leftovers drain right after the
        # branch, which still meets every deadline.
        GAPS = [
            # G0 (inside self0): V heads 0-5, then knl0's projections
            [("v", t, 0, 6) for t in range(8)]
            + [("kq", 0, 0), ("kq", 0, 1), ("kk", 0, 0)],
            # G1 (inside knl0): KV heads 0-5 for knl0's ctx flush
            [("kv", t, 0, 6) for t in range(4)],
            [("q", 1, 0), ("q", 1, 1)],                    # G2
            [("k", 1, 0), ("k", 1, 1)],                    # G3 (self2 needs jt1)
            [("kq", 1, 0), ("kq", 1, 1), ("kk", 1, 0)],    # G4 (knl2 needs jt1)
            [],                                            # G5
            [("q", 2, 0), ("q", 2, 1)],                    # G6
            [("k", 2, 0), ("k", 2, 1)],                    # G7 (self4 needs jt2)
            [("kq", 2, 0), ("kq", 2, 1), ("kk", 2, 0)],    # G8 (knl4 needs jt2)
            [],                                            # G9
            [("q", 3, 0), ("q", 3, 1)],                    # G10
            [("k", 3, 0), ("k", 3, 1)],                    # G11 (self6 needs jt3)
            # G12 (inside self6): kq jt3 + V head 6 (read by sflush(6))
            [("kq", 3, 0), ("kq", 3, 1)] + [("v", t, 6, 1) for t in range(8)],
            # G13 (inside knl6): KV head 6 (read by kflush(6) inside knl7)
            [("kv", t, 6, 1) for t in range(4)],
            # G14 (inside self7): kk jt3 (knl6 follows) + V head 7
            [("kk", 3, 0)] + [("v", t, 7, 1) for t in range(8)],
            # G15 (inside knl7): KV head 7 (read by the final kflush(7))
            [("kv", t, 7, 1) for t in range(4)],
        ]

        # ---- attention branches ----
        # `weave` is a list of callables (filler units, previous-branch ctx
        # flush parts, normalize/merge closures) emitted spread across the kt
        # steps so the PE always has ready work queued between ACT-dependent
        # matmuls.
        def head_branch(h, kt_mat, q_mat, vaug, n_keys, msk, weave,
                        split_last=False):
            base = (h % 2) * HD
            jt = h // 2
            nkt = n_keys // P
            ctxA = psctx.tile([P, 4, HD + 1], F32, name=f"cA_{h}_{n_keys}",
                              tag="ctxps")
            ctxB = psctx.tile([P, 4, HD + 1], F32, name=f"cB_{h}_{n_keys}",
                              tag="ctxps")

            nw = len(weave)
            slots = [min(nkt - 1, (j * nkt) // nw) for j in range(nw)]
            e_ts = []
            for kt in range(nkt):
                st_ps = psbig.tile([P, S], F32, name=f"st_{h}_{kt}", tag="big")
                for sc2 in range(S // 512):
                    nc.tensor.matmul(
                        st_ps[:, sc2 * 512:(sc2 + 1) * 512],
                        lhsT=kt_mat[base:base + HD, jt, kt * P:(kt + 1) * P],
                        rhs=q_mat[base:base + HD, jt, sc2 * 512:(sc2 + 1) * 512],
                        start=True, stop=True)
                e_t = epool.tile([P, S], BF16, name=f"e_{h}_{kt}", tag="e")
                if split_last and kt == nkt - 1:
                    # halve the last exp so the s<512 ctx groups (and the
                    # final normalize/merge/DMA chain) start half an exp early
                    for eh in range(2):
                        nc.scalar.activation(
                            e_t[:, eh * 512:(eh + 1) * 512],
                            st_ps[:, eh * 512:(eh + 1) * 512], AF.Exp,
                            bias=msk[:, kt:kt + 1], scale=INV)
                else:
                    nc.scalar.activation(e_t, st_ps, AF.Exp,
                                         bias=msk[:, kt:kt + 1], scale=INV)
                if _DUMP and h == 0 and n_keys == TKS:
                    d = nc.dram_tensor(f"d_e0s_{kt}", [P, S], F32,
                                       kind="ExternalOutput")
                    nc.gpsimd.dma_start(out=d.ap(), in_=e_t)
                e_ts.append(e_t)
                for j in range(nw):
                    if slots[j] == kt:
                        weave[j]()

            def flush_part(scs):
                # sc-outer / kt-inner: one open accumulation group per PSUM
                # region at a time (interleaved groups corrupt each other)
                for sc in scs:
                    t = ctxA if sc < 4 else ctxB
                    for kt in range(nkt):
                        nc.tensor.matmul(
                            t[:, sc % 4, :],
                            lhsT=e_ts[kt][:, sc * P:(sc + 1) * P],
                            rhs=vaug[:, kt, h, :],
                            start=(kt == 0), stop=(kt == nkt - 1))

            flush_parts = [lambda scs=(sc0, sc0 + 1): flush_part(scs)
                           for sc0 in range(0, S // P, 2)]
            return (ctxA, ctxB), flush_parts

        def self_branch(h, weave):
            return head_branch(h, KT, QT, Vaug, TKS, mask_sb, weave)

        def knl_branch(h, weave, split_last=False):
            return head_branch(h, KKT, KQT, KVaug, TKK, emask_sb, weave,
                               split_last)

        def norm_part(h, t, i, dst):
            # dst[:, 4i:4i+4, :] = ctx-tile / (2*denominator)
            rb = smallp.tile([P, 4, 1], F32, name=f"rb_{h}_{i}", tag="rb",
                             bufs=4)
            nc.vector.reciprocal(rb, t[:, :, HD:HD + 1])
            nc.vector.tensor_tensor(
                out=dst[:, i * 4:i * 4 + 4, :], in0=t[:, :, 0:HD],
                in1=rb.broadcast_to([P, 4, HD]), op=ALU.mult)

        def out_dma(q, half):
            # head-pair quarter q: heads 2q, 2q+1 -> out columns [128q, 128q+128),
            # s-half `half` -> rows [512*half, 512*half+512)
            oh = out_half[q // 2]
            hp = (2 * q) % 4
            nc.sync.dma_start(
                out=out.ap()[half * 512:(half + 1) * 512,
                             q * P:(q + 1) * P].rearrange(
                    "(sc p) j -> p sc j", p=P),
                in_=oh[:, half * 4:(half + 1) * 4, hp:hp + 2, :].rearrange(
                    "p sc h d -> p sc (h d)"))

        # ---- main schedule: self(h) then knl(h); each branch weaves in the
        # previous branch's ctx flush + normalize/merge plus this gap's
        # projection units, so the ACT-bound exp chain is always overlapped
        # with ready PE work ----
        def units(g):
            return [lambda u=u: fill_unit(u) for u in GAPS[g]]

        # the last four branches run self-self-knl-knl: the knowledge
        # branches have a smaller exp-vs-PE deficit, so putting them at the
        # end (where no projection filler remains) shrinks the tail idle
        schedule = []
        for h in range(6):
            schedule += [("s", h), ("k", h)]
        schedule += [("s", 6), ("s", 7), ("k", 6), ("k", 7)]

        carry = []   # work woven into the next branch
        sN = {}
        for kind, h in schedule:
            if kind == "s":
                ctx_s, sfl = self_branch(h, carry + units(2 * h))

                # flush parts interleaved with the normalize halves they feed
                def mk_sn(i, h=h, ctx_s=ctx_s):
                    def f():
                        if i == 0:
                            sN[h] = snpool.tile([P, S // P, HD], F32,
                                                name=f"sN_{h}", tag="sN")
                        norm_part(h, ctx_s[i], i, sN[h])
                    return f

                carry = [sfl[0], sfl[1], mk_sn(0), sfl[2], sfl[3], mk_sn(1)]
            else:
                ctx_k, kfl = knl_branch(h, carry + units(2 * h + 1),
                                        split_last=(h == 7))

                tmp = {}

                def mk_k(i, h=h, ctx_k=ctx_k, tmp=tmp):
                    def f():
                        if i == 0:
                            tmp[0] = smallp.tile([P, S // P, HD], F32,
                                                 name=f"tK_{h}", tag="tK")
                        norm_part(h + 8, ctx_k[i], i, tmp[0])
                        oh = out_half[h // 4]
                        nc.vector.tensor_tensor(
                            out=oh[:, i * 4:i * 4 + 4, h % 4, :],
                            in0=tmp[0][:, i * 4:i * 4 + 4, :],
                            in1=sN[h][:, i * 4:i * 4 + 4, :], op=ALU.add)
                        if i == 1:
                            sN.pop(h)
                    return f

                carry = [kfl[0], kfl[1], mk_k(0), kfl[2], kfl[3], mk_k(1)]
                if h % 2 == 1:
                    carry.insert(3, lambda q=h // 2: out_dma(q, 0))
                    carry.append(lambda q=h // 2: out_dma(q, 1))
        for f in carry:
            f()

    nc.finalize()
    return nc


def _get_nc():
    if "nc" not in _CACHE:
        _CACHE["nc"] = _build()
    return _CACHE["nc"]


def kernel(**inputs):
    inp = {k: np.asarray(v, dtype=np.float32) for k, v in inputs.items()}
    nc = _get_nc()

    def bf16(x):
        return np.ascontiguousarray(x.astype(ml_dtypes.bfloat16))

    B = 4
    in_maps = []
    for core in range(8):
        b, hg = core // 2, core % 2
        sl = slice(hg * HG, (hg + 1) * HG)
        m = {
            "hsT": bf16(inp["hidden_states"][b].T),
            "ehsT": bf16(inp["encoder_hidden_states"][b].T),
            "mask": np.ascontiguousarray(inp["attention_mask"][b, 0, 0, :]),
            "emask": np.ascontiguousarray(inp["encoder_attention_mask"][b, 0, 0, :]),
        }
        for nm in ["q", "k", "v", "kq", "kk", "kv"]:
            m[f"w{nm}"] = bf16(inp[f"W{nm}"][:, sl])
            m[f"b{nm}"] = np.ascontiguousarray(inp[f"b{nm}"][sl])
        in_maps.append(m)

    res = run_bass_kernel_spmd(nc, in_maps, core_ids=list(range(8)))

    outp = np.empty((B, S, H), np.float32)
    for core in range(8):
        b, hg = core // 2, core % 2
        outp[b, :, hg * HG:(hg + 1) * HG] = res.results[core]["out"]
    return outp


# revision 71
# speedup vs baseline: 1.0080x; 1.0080x over previous
"""Trainium2 Bass kernel for nn_BertSelfAttention_7962869367489.

Dual-branch (self + cross/"knowledge") BERT attention, B=4, S=1024, K=512,
H=1024, NH=16, HD=64, fp32.

Sharding: 8 cores = (batch b in 0..3) x (head-group hg in 0..1, 8 heads each).
All six projections are column-split by head-group; per-head attention is
entirely core-local; output columns are disjoint per core, so the gather is a
pure concatenation (no collectives).

Per-core pipeline (bf16 operands everywhere on the PE; f32 PSUM accumulation):
  - hs/ehs arrive pre-transposed and pre-cast to bf16 from the host
    (hsT [H,S], ehsT [H,K]), weights pre-cast to bf16, so no on-device
    transposes or casts are needed and input DMA bytes are halved.
  - Projections: QT/KT/KQT/KKT = W.T @ srcT in transposed orientation (bf16
    outs); Vaug/KVaug = srcT.T @ Wv in normal orientation with an augmented
    column of 2.0, so the ctx matmul also produces 2*softmax-denominator,
    folding the (ctx+kctx)*0.5 branch average into the normalization.
  - Per head h: scoresT[t,s] = K_h @ Q_h^T (contraction HD=64); exp on ACT
    with per-partition mask bias and 1/8 scale, written bf16; after the
    branch's exps, ctx[s,d|den] accumulates in PSUM in NORMAL orientation
    via lhsT = e-chunk [t,128s], rhs = Vaug_h [t,65] -- 65-row bf16 matmuls,
    ~2.4x fewer PE rows than the transposed form and no output transposes.
    Accumulation groups run sc-outer/kt-inner so each PSUM region hosts one
    group at a time (interleaved groups in one bank clobber each other).
  - Normalization + branch merge on DVE straight out of PSUM; output DMA'd
    in four head-pair quarters.
  - Remaining projections are split into ~1.7us (jt, sc) units and drained
    between attention branches so PE projection work fills the ACT-bound
    exp windows; knowledge branch h runs right after self branch h.
"""
import numpy as np
import ml_dtypes
from contextlib import ExitStack

import concourse.bacc as bacc
import concourse.tile as tile
import concourse.mybir as mybir
from concourse.bass_utils import run_bass_kernel_spmd

F32 = mybir.dt.float32
BF16 = mybir.dt.bfloat16
AF = mybir.ActivationFunctionType
ALU = mybir.AluOpType

P = 128
S = 1024        # query length
TKS = 1024      # self-branch key length
TKK = 512       # knowledge-branch key length
H = 1024        # model dim (projection contraction)
HG = 512        # per-core output width (8 heads x 64)
NHL = 8         # heads per core
HD = 64
HC = H // P     # 8 contraction chunks
INV = 0.125     # 1/sqrt(64)

_CACHE = {}
_DUMP = False


def _build():
    nc = bacc.Bacc(target_bir_lowering=False, debug=False)

    hsT = nc.dram_tensor("hsT", [H, S], BF16, kind="ExternalInput")
    ehsT = nc.dram_tensor("ehsT", [H, TKK], BF16, kind="ExternalInput")
    # host-packed [wq jt0-cols | wk jt0-cols]: contiguous 512B rows dodge the
    # 256B-run DMA penalty on the startup-critical first weight loads
    wqk0 = nc.dram_tensor("wqk0", [H, 2 * P], BF16, kind="ExternalInput")
    w_in = {}
    b_in = {}
    for nm in ["q", "k", "v", "kq", "kk", "kv"]:
        w_in[nm] = nc.dram_tensor(f"w{nm}", [H, HG], BF16, kind="ExternalInput")
        b_in[nm] = nc.dram_tensor(f"b{nm}", [HG], F32, kind="ExternalInput")
    mask = nc.dram_tensor("mask", [TKS], F32, kind="ExternalInput")
    emask = nc.dram_tensor("emask", [TKK], F32, kind="ExternalInput")
    out = nc.dram_tensor("out", [S, HG], F32, kind="ExternalOutput")

    with tile.TileContext(nc) as tc, ExitStack() as ctx:
        const = ctx.enter_context(tc.tile_pool(name="const", bufs=1))
        persist = ctx.enter_context(tc.tile_pool(name="persist", bufs=1))
        epool = ctx.enter_context(tc.tile_pool(name="epool", bufs=17))
        smallp = ctx.enter_context(tc.tile_pool(name="smallp", bufs=2))
        snpool = ctx.enter_context(tc.tile_pool(name="snpool", bufs=3))
        psproj = ctx.enter_context(tc.tile_pool(name="psproj", bufs=2, space="PSUM"))
        psbig = ctx.enter_context(tc.tile_pool(name="psbig", bufs=2, space="PSUM"))
        psctx = ctx.enter_context(tc.tile_pool(name="psctx", bufs=2, space="PSUM"))

        # ---- constants (gpsimd/SWDGE queue, but the DMA engines are shared,
        # so these are emitted interleaved with the big loads below in
        # need-order to keep the startup-critical stream dense) ----
        mask_sb = const.tile([P, TKS // P], F32)
        emask_sb = const.tile([P, TKK // P], F32)
        bias_col = {}
        for nm in ["q", "k", "kq", "kk"]:
            bias_col[nm] = const.tile([P, 4], F32, name=f"bias_{nm}")
        bias_row = {}
        for nm in ["v", "kv"]:
            bias_row[nm] = const.tile([P, HG], F32, name=f"brow_{nm}")
        twos = const.tile([P, 1], F32)
        nc.vector.memset(twos, 2.0)

        def load_consts_early():
            for nm in ["q", "k"]:
                nc.sync.dma_start(
                    out=bias_col[nm],
                    in_=b_in[nm].ap().rearrange("(jt p) -> p jt", p=P))
            nc.sync.dma_start(out=mask_sb,
                              in_=mask.ap().rearrange("(kt p) -> p kt", p=P))

        def load_consts_late():
            nc.sync.dma_start(out=emask_sb,
                              in_=emask.ap().rearrange("(kt p) -> p kt", p=P))
            for nm in ["kq", "kk"]:
                nc.sync.dma_start(
                    out=bias_col[nm],
                    in_=b_in[nm].ap().rearrange("(jt p) -> p jt", p=P))

        # ---- persistent activations ----
        QT = persist.tile([P, 4, S], BF16)       # [j%128, jt, s]
        KT = persist.tile([P, 4, TKS], BF16)
        KQT = persist.tile([P, 4, S], BF16)
        KKT = persist.tile([P, 4, TKK], BF16)
        Vaug = persist.tile([P, TKS // P, NHL, HD + 1], BF16)   # [t%128, tt, h, d|2]
        KVaug = persist.tile([P, TKK // P, NHL, HD + 1], BF16)
        hsT_sb = persist.tile([P, HC, S], BF16)   # [h%128, hc, s]
        ehsT_sb = persist.tile([P, HC, TKK], BF16)
        wsb = {}
        for nm in ["q", "k", "v", "kq", "kk", "kv"]:
            wsb[nm] = persist.tile([P, HC, HG], BF16, name=f"w_{nm}")
        wqk0_sb = persist.tile([P, HC, 2 * P], BF16)
        # output staging in two head-halves; DMA'd in four head-pair quarters
        out_half = [persist.tile([P, S // P, 4, HD], F32, name=f"out_half{i}",
                                 tag=f"out_half{i}") for i in range(2)]

        # ---- input DMAs (sync/HWDGE queue), ordered so the prelude's
        # dependencies (hsT, wq, wk) land first ----
        def load_rows(dst, src, half, rows, cols):
            nc.sync.dma_start(
                out=dst[:, half * (rows // 2):(half + 1) * (rows // 2), :],
                in_=src[half * (rows * P // 2):(half + 1) * (rows * P // 2), :]
                .rearrange("(hc p) s -> p hc s", p=P))

        def load_w(nm, hc0, hcn):
            nc.sync.dma_start(
                out=wsb[nm][:, hc0:hc0 + hcn, :],
                in_=w_in[nm][hc0 * P:(hc0 + hcn) * P, :].rearrange(
                    "(hc p) j -> p hc j", p=P))

        def load_wqk0(hc0, hcn):
            nc.sync.dma_start(
                out=wqk0_sb[:, hc0:hc0 + hcn, :],
                in_=wqk0[hc0 * P:(hc0 + hcn) * P, :].rearrange(
                    "(hc p) j -> p hc j", p=P))

        # startup-critical loads (prelude needs wq/wk jt0 + all of hsT),
        # split fine and interleaved so the first projection matmuls start
        # (and the PE p-state ramps) as early as possible
        load_wqk0(0, 2)
        nc.sync.dma_start(out=hsT_sb[:, 0:1, :], in_=hsT[0:P, :].rearrange(
            "(hc p) s -> p hc s", p=P))
        load_wqk0(2, 2)
        nc.sync.dma_start(out=hsT_sb[:, 1:2, :], in_=hsT[P:2 * P, :].rearrange(
            "(hc p) s -> p hc s", p=P))
        load_wqk0(4, 2)
        load_rows(hsT_sb, hsT, 1, HC // 2, S)   # hc 2-3
        load_wqk0(6, 2)
        load_rows(hsT_sb, hsT, 2, HC // 2, S)   # hc 4-5
        load_rows(hsT_sb, hsT, 3, HC // 2, S)   # hc 6-7
        load_consts_early()
        load_w("v", 0, HC)
        nc.sync.dma_start(out=bias_row["v"],
                           in_=b_in["v"].ap().unsqueeze(0).broadcast_to([P, HG]))
        load_w("kq", 0, HC)
        load_w("kk", 0, HC)
        load_rows(ehsT_sb, ehsT, 0, HC, TKK)
        load_rows(ehsT_sb, ehsT, 1, HC, TKK)
        load_w("kv", 0, HC)
        load_consts_late()
        # the remaining wq/wk column blocks (jt1-3)
        def load_w_jt13(nm):
            nc.sync.dma_start(
                out=wsb[nm][:, :, P:4 * P],
                in_=w_in[nm][:, P:4 * P].rearrange("(hc p) j -> p hc j", p=P))

        load_w_jt13("q")
        load_w_jt13("k")
        nc.sync.dma_start(out=bias_row["kv"],
                           in_=b_in["kv"].ap().unsqueeze(0).broadcast_to([P, HG]))

        # ---- projection emitters ----
        def proj_t_unit(nm, dst, srcT, jt, sc, nsc):
            """One (jt, sc) unit: 8 hc-chunk matmuls + bias-add."""
            w = 512 if nsc > 1 else TKK
            ps = psproj.tile([P, w], F32, name="psj", tag="psj")
            for hc in range(HC):
                nc.tensor.matmul(
                    ps, lhsT=wsb[nm][:, hc, jt * P:(jt + 1) * P],
                    rhs=srcT[:, hc, sc * w:(sc + 1) * w],
                    start=(hc == 0), stop=(hc == HC - 1))
            nc.vector.tensor_scalar_add(
                dst[:, jt, sc * w:(sc + 1) * w], ps,
                bias_col[nm][:, jt:jt + 1])

        def proj_v_part(nm, dst, srcT, tt, h0, nh):
            # V-projection for a head subrange: each attention branch reads
            # only its own head's Vaug column, so the head-6/7 parts carry
            # end-of-schedule deadlines and can fill the tail gaps
            ps = psproj.tile([P, nh * HD], F32, name=f"psv{tt}_{h0}",
                             tag="psj")
            for hc in range(HC):
                nc.tensor.matmul(
                    ps, lhsT=srcT[:, hc, tt * P:(tt + 1) * P],
                    rhs=wsb[nm][:, hc, h0 * HD:(h0 + nh) * HD],
                    start=(hc == 0), stop=(hc == HC - 1))
            nc.vector.scalar_tensor_tensor(
                out=dst[:, tt, h0:h0 + nh, 0:HD],
                in0=ps.rearrange("p (h d) -> p h d", h=nh),
                scalar=1.0,
                in1=bias_row[nm].rearrange(
                    "p (h d) -> p h d", h=NHL)[:, h0:h0 + nh, :],
                op0=ALU.mult, op1=ALU.add)
            nc.vector.tensor_copy(
                dst[:, tt, h0:h0 + nh, HD:HD + 1],
                twos.unsqueeze(1).broadcast_to([P, nh, 1]))

        # ---- prelude: Q/K jt0 with all four (proj, sc) accumulation groups
        # concurrent (two in the idle scores-psum slots), matmuls woven in
        # DMA-arrival order so the PE starts and p-state-ramps early ----
        pre_ps = {
            ("q", 0): psproj.tile([P, 512], F32, name="pre_q0", tag="psj"),
            ("k", 0): psproj.tile([P, 512], F32, name="pre_k0", tag="psj"),
            ("q", 1): psctx.tile([P, 512], F32, name="pre_q1", tag="ctxps"),
            ("k", 1): psctx.tile([P, 512], F32, name="pre_k1", tag="ctxps"),
        }
        for hc2 in range(HC // 2):
            for nm in ["q", "k"]:
                off = 0 if nm == "q" else P
                for hc in (2 * hc2, 2 * hc2 + 1):
                    for sc in range(2):
                        nc.tensor.matmul(
                            pre_ps[(nm, sc)],
                            lhsT=wqk0_sb[:, hc, off:off + P],
                            rhs=hsT_sb[:, hc, sc * 512:(sc + 1) * 512],
                            start=(hc == 0), stop=(hc == HC - 1))
        # bias-adds ordered so the first scores matmul's inputs (QT s-half 0
        # and KT key-half 0) complete first
        for nm, sc in [("q", 0), ("k", 0), ("q", 1), ("k", 1)]:
            dst = QT if nm == "q" else KT
            nc.vector.tensor_scalar_add(
                dst[:, 0, sc * 512:(sc + 1) * 512], pre_ps[(nm, sc)],
                bias_col[nm][:, 0:1])

        # ---- filler units: remaining projections, drained between branches
        def fill_unit(u):
            nm, jt, sc = u[0], u[1], u[2]
            if nm == "v":
                proj_v_part("v", Vaug, hsT_sb, u[1], u[2], u[3])
            elif nm == "kv":
                proj_v_part("kv", KVaug, ehsT_sb, u[1], u[2], u[3])
            elif nm == "kk":
                proj_t_unit("kk", KKT, ehsT_sb, jt, 0, 1)
            elif nm == "q":
                proj_t_unit("q", QT, hsT_sb, jt, sc, 2)
            elif nm == "k":
                proj_t_unit("k", KT, hsT_sb, jt, sc, 2)
            elif nm == "kq":
                proj_t_unit("kq", KQT, hsT_sb, jt, sc, 2)

        # gap -> units (after self(h) -> G(2h), after knl(h) -> G(2h+1)).
        # Units are woven INSIDE the following branch's kt loop (paced across
        # the kt steps) so ready filler work sits between the ACT-dependent
        # scores/ctx matmuls in the PE queue; leftovers drain right after the
        # branch, which still meets every deadline.
        GAPS = [
            # G0 (inside self0): V heads 0-5, then knl0's projections
            [("v", t, 0, 6) for t in range(8)]
            + [("kq", 0, 0), ("kq", 0, 1), ("kk", 0, 0)],
            # G1 (inside knl0): KV heads 0-5 for knl0's ctx flush
            [("kv", t, 0, 6) for t in range(4)],
            [("q", 1, 0), ("q", 1, 1)],                    # G2
            [("k", 1, 0), ("k", 1, 1)],                    # G3 (self2 needs jt1)
            [("kq", 1, 0), ("kq", 1, 1), ("kk", 1, 0)],    # G4 (knl2 needs jt1)
            [],                                            # G5
            [("q", 2, 0), ("q", 2, 1)],                    # G6
            [("k", 2, 0), ("k", 2, 1)],                    # G7 (self4 needs jt2)
            [("kq", 2, 0), ("kq", 2, 1), ("kk", 2, 0)],    # G8 (knl4 needs jt2)
            [],                                            # G9
            [("q", 3, 0), ("q", 3, 1)],                    # G10
            [("k", 3, 0), ("k", 3, 1)],                    # G11 (self6 needs jt3)
            # G12 (inside self6): kq jt3 + V head 6 (read by sflush(6))
            [("kq", 3, 0), ("kq", 3, 1)] + [("v", t, 6, 1) for t in range(8)],
            # G13 (inside knl6): KV head 6 (read by kflush(6) inside knl7)
            [("kv", t, 6, 1) for t in range(4)],
            # G14 (inside self7): kk jt3 (knl6 follows) + V head 7
            [("kk", 3, 0)] + [("v", t, 7, 1) for t in range(8)],
            # G15 (inside knl7): KV head 7 (read by the final kflush(7))
            [("kv", t, 7, 1) for t in range(4)],
        ]

        # ---- attention branches ----
        # `weave` is a list of callables (filler units, previous-branch ctx
        # flush parts, normalize/merge closures) emitted spread across the kt
        # steps so the PE always has ready work queued between ACT-dependent
        # matmuls.
        def head_branch(h, kt_mat, q_mat, vaug, n_keys, msk, weave,
                        split_last=False):
            base = (h % 2) * HD
            jt = h // 2
            nkt = n_keys // P
            ctxA = psctx.tile([P, 4, HD + 1], F32, name=f"cA_{h}_{n_keys}",
                              tag="ctxps")
            ctxB = psctx.tile([P, 4, HD + 1], F32, name=f"cB_{h}_{n_keys}",
                              tag="ctxps")

            nw = len(weave)
            slots = [min(nkt - 1, (j * nkt) // nw) for j in range(nw)]
            e_ts = []
            for kt in range(nkt):
                st_ps = psbig.tile([P, S], F32, name=f"st_{h}_{kt}", tag="big")
                for sc2 in range(S // 512):
                    nc.tensor.matmul(
                        st_ps[:, sc2 * 512:(sc2 + 1) * 512],
                        lhsT=kt_mat[base:base + HD, jt, kt * P:(kt + 1) * P],
                        rhs=q_mat[base:base + HD, jt, sc2 * 512:(sc2 + 1) * 512],
                        start=True, stop=True)
                e_t = epool.tile([P, S], BF16, name=f"e_{h}_{kt}", tag="e")
                if split_last and kt == nkt - 1:
                    # halve the last exp so the s<512 ctx groups (and the
                    # final normalize/merge/DMA chain) start half an exp early
                    for eh in range(2):
                        nc.scalar.activation(
                            e_t[:, eh * 512:(eh + 1) * 512],
                            st_ps[:, eh * 512:(eh + 1) * 512], AF.Exp,
                            bias=msk[:, kt:kt + 1], scale=INV)
                else:
                    nc.scalar.activation(e_t, st_ps, AF.Exp,
                                         bias=msk[:, kt:kt + 1], scale=INV)
                if _DUMP and h == 0 and n_keys == TKS:
                    d = nc.dram_tensor(f"d_e0s_{kt}", [P, S], F32,
                                       kind="ExternalOutput")
                    nc.gpsimd.dma_start(out=d.ap(), in_=e_t)
                e_ts.append(e_t)
                for j in range(nw):
                    if slots[j] == kt:
                        weave[j]()

            def flush_part(scs):
                # sc-outer / kt-inner: one open accumulation group per PSUM
                # region at a time (interleaved groups corrupt each other)
                for sc in scs:
                    t = ctxA if sc < 4 else ctxB
                    for kt in range(nkt):
                        nc.tensor.matmul(
                            t[:, sc % 4, :],
                            lhsT=e_ts[kt][:, sc * P:(sc + 1) * P],
                            rhs=vaug[:, kt, h, :],
                            start=(kt == 0), stop=(kt == nkt - 1))

            flush_parts = [lambda scs=(sc0, sc0 + 1): flush_part(scs)
                           for sc0 in range(0, S // P, 2)]
            return (ctxA, ctxB), flush_parts

        def self_branch(h, weave):
            return head_branch(h, KT, QT, Vaug, TKS, mask_sb, weave)

        def knl_branch(h, weave, split_last=False):
            return head_branch(h, KKT, KQT, KVaug, TKK, emask_sb, weave,
                               split_last)

        def norm_part(h, t, i, dst):
            # dst[:, 4i:4i+4, :] = ctx-tile / (2*denominator)
            rb = smallp.tile([P, 4, 1], F32, name=f"rb_{h}_{i}", tag="rb",
                             bufs=4)
            nc.vector.reciprocal(rb, t[:, :, HD:HD + 1])
            nc.vector.tensor_tensor(
                out=dst[:, i * 4:i * 4 + 4, :], in0=t[:, :, 0:HD],
                in1=rb.broadcast_to([P, 4, HD]), op=ALU.mult)

        def out_dma(q, half):
            # head-pair quarter q: heads 2q, 2q+1 -> out columns [128q, 128q+128),
            # s-half `half` -> rows [512*half, 512*half+512)
            oh = out_half[q // 2]
            hp = (2 * q) % 4
            nc.sync.dma_start(
                out=out.ap()[half * 512:(half + 1) * 512,
                             q * P:(q + 1) * P].rearrange(
                    "(sc p) j -> p sc j", p=P),
                in_=oh[:, half * 4:(half + 1) * 4, hp:hp + 2, :].rearrange(
                    "p sc h d -> p sc (h d)"))

        # ---- main schedule: self(h) then knl(h); each branch weaves in the
        # previous branch's ctx flush + normalize/merge plus this gap's
        # projection units, so the ACT-bound exp chain is always overlapped
        # with ready PE work ----
        def units(g):
            return [lambda u=u: fill_unit(u) for u in GAPS[g]]

        # the last four branches run self-self-knl-knl: the knowledge
        # branches have a smaller exp-vs-PE deficit, so putting them at the
        # end (where no projection filler remains) shrinks the tail idle
        schedule = []
        for h in range(6):
            schedule += [("s", h), ("k", h)]
        schedule += [("s", 6), ("s", 7), ("k", 6), ("k", 7)]

        carry = []   # work woven into the next branch
        sN = {}
        for kind, h in schedule:
            if kind == "s":
                ctx_s, sfl = self_branch(h, carry + units(2 * h))

                # flush parts interleaved with the normalize halves they feed
                def mk_sn(i, h=h, ctx_s=ctx_s):
                    def f():
                        if i == 0:
                            sN[h] = snpool.tile([P, S // P, HD], F32,
                                                name=f"sN_{h}", tag="sN")
                        norm_part(h, ctx_s[i], i, sN[h])
                    return f

                carry = [sfl[0], sfl[1], mk_sn(0), sfl[2], sfl[3], mk_sn(1)]
            else:
                ctx_k, kfl = knl_branch(h, carry + units(2 * h + 1),
                                        split_last=(h == 7))

                tmp = {}

                def mk_k(i, h=h, ctx_k=ctx_k, tmp=tmp):
                    def f():
                        if i == 0:
                            tmp[0] = smallp.tile([P, S // P, HD], F32,
                                                 name=f"tK_{h}", tag="tK")
                        norm_part(h + 8, ctx_k[i], i, tmp[0])
                        oh = out_half[h // 4]
                        nc.vector.tensor_tensor(
                            out=oh[:, i * 4:i * 4 + 4, h % 4, :],
                            in0=tmp[0][:, i * 4:i * 4 + 4, :],
                            in1=sN[h][:, i * 4:i * 4 + 4, :], op=ALU.add)
                        if i == 1:
                            sN.pop(h)
                    return f

                carry = [kfl[0], kfl[1], mk_k(0), kfl[2], kfl[3], mk_k(1)]
                if h % 2 == 1:
                    carry.insert(3, lambda q=h // 2: out_dma(q, 0))
                    carry.append(lambda q=h // 2: out_dma(q, 1))
        for f in carry:
            f()

    nc.finalize()
    return nc


def _get_nc():
    if "nc" not in _CACHE:
        _CACHE["nc"] = _build()
    return _CACHE["nc"]


def kernel(**inputs):
    inp = {k: np.asarray(v, dtype=np.float32) for k, v in inputs.items()}
    nc = _get_nc()

    def bf16(x):
        return np.ascontiguousarray(x.astype(ml_dtypes.bfloat16))

    B = 4
    in_maps = []
    for core in range(8):
        b, hg = core // 2, core % 2
        sl = slice(hg * HG, (hg + 1) * HG)
        m = {
            "hsT": bf16(inp["hidden_states"][b].T),
            "ehsT": bf16(inp["encoder_hidden_states"][b].T),
            "wqk0": bf16(np.concatenate(
                [inp["Wq"][:, sl][:, 0:P], inp["Wk"][:, sl][:, 0:P]], axis=1)),
            "mask": np.ascontiguousarray(inp["attention_mask"][b, 0, 0, :]),
            "emask": np.ascontiguousarray(inp["encoder_attention_mask"][b, 0, 0, :]),
        }
        for nm in ["q", "k", "v", "kq", "kk", "kv"]:
            m[f"w{nm}"] = bf16(inp[f"W{nm}"][:, sl])
            m[f"b{nm}"] = np.ascontiguousarray(inp[f"b{nm}"][sl])
        in_maps.append(m)

    res = run_bass_kernel_spmd(nc, in_maps, core_ids=list(range(8)))

    outp = np.empty((B, S, H), np.float32)
    for core in range(8):
        b, hg = core // 2, core % 2
        outp[b, :, hg * HG:(hg + 1) * HG] = res.results[core]["out"]
    return outp


# revision 72
# speedup vs baseline: 1.0112x; 1.0032x over previous
"""Trainium2 Bass kernel for nn_BertSelfAttention_7962869367489.

Dual-branch (self + cross/"knowledge") BERT attention, B=4, S=1024, K=512,
H=1024, NH=16, HD=64, fp32.

Sharding: 8 cores = (batch b in 0..3) x (head-group hg in 0..1, 8 heads each).
All six projections are column-split by head-group; per-head attention is
entirely core-local; output columns are disjoint per core, so the gather is a
pure concatenation (no collectives).

Per-core pipeline (bf16 operands everywhere on the PE; f32 PSUM accumulation):
  - hs/ehs arrive pre-transposed and pre-cast to bf16 from the host
    (hsT [H,S], ehsT [H,K]), weights pre-cast to bf16, so no on-device
    transposes or casts are needed and input DMA bytes are halved.
  - Projections: QT/KT/KQT/KKT = W.T @ srcT in transposed orientation (bf16
    outs); Vaug/KVaug = srcT.T @ Wv in normal orientation with an augmented
    column of 2.0, so the ctx matmul also produces 2*softmax-denominator,
    folding the (ctx+kctx)*0.5 branch average into the normalization.
  - Per head h: scoresT[t,s] = K_h @ Q_h^T (contraction HD=64); exp on ACT
    with per-partition mask bias and 1/8 scale, written bf16; after the
    branch's exps, ctx[s,d|den] accumulates in PSUM in NORMAL orientation
    via lhsT = e-chunk [t,128s], rhs = Vaug_h [t,65] -- 65-row bf16 matmuls,
    ~2.4x fewer PE rows than the transposed form and no output transposes.
    Accumulation groups run sc-outer/kt-inner so each PSUM region hosts one
    group at a time (interleaved groups in one bank clobber each other).
  - Normalization + branch merge on DVE straight out of PSUM; output DMA'd
    in four head-pair quarters.
  - Remaining projections are split into ~1.7us (jt, sc) units and drained
    between attention branches so PE projection work fills the ACT-bound
    exp windows; knowledge branch h runs right after self branch h.
"""
import numpy as np
import ml_dtypes
from contextlib import ExitStack

import concourse.bacc as bacc
import concourse.tile as tile
import concourse.mybir as mybir
from concourse.bass_utils import run_bass_kernel_spmd

F32 = mybir.dt.float32
BF16 = mybir.dt.bfloat16
AF = mybir.ActivationFunctionType
ALU = mybir.AluOpType

P = 128
S = 1024        # query length
TKS = 1024      # self-branch key length
TKK = 512       # knowledge-branch key length
H = 1024        # model dim (projection contraction)
HG = 512        # per-core output width (8 heads x 64)
NHL = 8         # heads per core
HD = 64
HC = H // P     # 8 contraction chunks
INV = 0.125     # 1/sqrt(64)

_CACHE = {}
_DUMP = False


def _build():
    nc = bacc.Bacc(target_bir_lowering=False, debug=False)

    hsT = nc.dram_tensor("hsT", [H, S], BF16, kind="ExternalInput")
    ehsT = nc.dram_tensor("ehsT", [H, TKK], BF16, kind="ExternalInput")
    # host-packed [wq jt0-cols | wk jt0-cols]: contiguous 512B rows dodge the
    # 256B-run DMA penalty on the startup-critical first weight loads
    wqk0 = nc.dram_tensor("wqk0", [H, 2 * P], BF16, kind="ExternalInput")
    w_in = {}
    b_in = {}
    for nm in ["q", "k", "v", "kq", "kk", "kv"]:
        w_in[nm] = nc.dram_tensor(f"w{nm}", [H, HG], BF16, kind="ExternalInput")
        b_in[nm] = nc.dram_tensor(f"b{nm}", [HG], F32, kind="ExternalInput")
    mask = nc.dram_tensor("mask", [TKS], F32, kind="ExternalInput")
    emask = nc.dram_tensor("emask", [TKK], F32, kind="ExternalInput")
    out = nc.dram_tensor("out", [S, HG], F32, kind="ExternalOutput")

    with tile.TileContext(nc) as tc, ExitStack() as ctx:
        const = ctx.enter_context(tc.tile_pool(name="const", bufs=1))
        persist = ctx.enter_context(tc.tile_pool(name="persist", bufs=1))
        epool = ctx.enter_context(tc.tile_pool(name="epool", bufs=17))
        smallp = ctx.enter_context(tc.tile_pool(name="smallp", bufs=2))
        snpool = ctx.enter_context(tc.tile_pool(name="snpool", bufs=3))
        psproj = ctx.enter_context(tc.tile_pool(name="psproj", bufs=2, space="PSUM"))
        psbig = ctx.enter_context(tc.tile_pool(name="psbig", bufs=2, space="PSUM"))
        psctx = ctx.enter_context(tc.tile_pool(name="psctx", bufs=2, space="PSUM"))

        # ---- constants (gpsimd/SWDGE queue, but the DMA engines are shared,
        # so these are emitted interleaved with the big loads below in
        # need-order to keep the startup-critical stream dense) ----
        mask_sb = const.tile([P, TKS // P], F32)
        emask_sb = const.tile([P, TKK // P], F32)
        bias_col = {}
        for nm in ["q", "k", "kq", "kk"]:
            bias_col[nm] = const.tile([P, 4], F32, name=f"bias_{nm}")
        bias_row = {}
        for nm in ["v", "kv"]:
            bias_row[nm] = const.tile([P, HG], F32, name=f"brow_{nm}")
        twos = const.tile([P, 1], F32)
        nc.vector.memset(twos, 2.0)

        def load_consts_early():
            for nm in ["q", "k"]:
                nc.sync.dma_start(
                    out=bias_col[nm],
                    in_=b_in[nm].ap().rearrange("(jt p) -> p jt", p=P))
            nc.sync.dma_start(out=mask_sb,
                              in_=mask.ap().rearrange("(kt p) -> p kt", p=P))

        def load_consts_late():
            nc.sync.dma_start(out=emask_sb,
                              in_=emask.ap().rearrange("(kt p) -> p kt", p=P))
            for nm in ["kq", "kk"]:
                nc.sync.dma_start(
                    out=bias_col[nm],
                    in_=b_in[nm].ap().rearrange("(jt p) -> p jt", p=P))

        # ---- persistent activations ----
        QT = persist.tile([P, 4, S], BF16)       # [j%128, jt, s]
        KT = persist.tile([P, 4, TKS], BF16)
        KQT = persist.tile([P, 4, S], BF16)
        KKT = persist.tile([P, 4, TKK], BF16)
        Vaug = persist.tile([P, TKS // P, NHL, HD + 1], BF16)   # [t%128, tt, h, d|2]
        KVaug = persist.tile([P, TKK // P, NHL, HD + 1], BF16)
        hsT_sb = persist.tile([P, HC, S], BF16)   # [h%128, hc, s]
        ehsT_sb = persist.tile([P, HC, TKK], BF16)
        wsb = {}
        for nm in ["q", "k", "v", "kq", "kk", "kv"]:
            wsb[nm] = persist.tile([P, HC, HG], BF16, name=f"w_{nm}")
        wqk0_sb = persist.tile([P, HC, 2 * P], BF16)
        # output staging in two head-halves; DMA'd in four head-pair quarters
        out_half = [persist.tile([P, S // P, 4, HD], F32, name=f"out_half{i}",
                                 tag=f"out_half{i}") for i in range(2)]

        # ---- input DMAs (sync/HWDGE queue), ordered so the prelude's
        # dependencies (hsT, wq, wk) land first ----
        def load_rows(dst, src, half, rows, cols):
            nc.sync.dma_start(
                out=dst[:, half * (rows // 2):(half + 1) * (rows // 2), :],
                in_=src[half * (rows * P // 2):(half + 1) * (rows * P // 2), :]
                .rearrange("(hc p) s -> p hc s", p=P))

        def load_w(nm, hc0, hcn):
            nc.sync.dma_start(
                out=wsb[nm][:, hc0:hc0 + hcn, :],
                in_=w_in[nm][hc0 * P:(hc0 + hcn) * P, :].rearrange(
                    "(hc p) j -> p hc j", p=P))

        def load_wqk0(hc0, hcn):
            nc.sync.dma_start(
                out=wqk0_sb[:, hc0:hc0 + hcn, :],
                in_=wqk0[hc0 * P:(hc0 + hcn) * P, :].rearrange(
                    "(hc p) j -> p hc j", p=P))

        # startup-critical loads (prelude needs wq/wk jt0 + all of hsT),
        # split fine and interleaved so the first projection matmuls start
        # (and the PE p-state ramps) as early as possible
        load_wqk0(0, 2)
        nc.sync.dma_start(out=hsT_sb[:, 0:1, :], in_=hsT[0:P, :].rearrange(
            "(hc p) s -> p hc s", p=P))
        load_wqk0(2, 2)
        nc.sync.dma_start(out=hsT_sb[:, 1:2, :], in_=hsT[P:2 * P, :].rearrange(
            "(hc p) s -> p hc s", p=P))
        load_wqk0(4, 2)
        load_rows(hsT_sb, hsT, 1, HC // 2, S)   # hc 2-3
        load_wqk0(6, 2)
        load_rows(hsT_sb, hsT, 2, HC // 2, S)   # hc 4-5
        load_rows(hsT_sb, hsT, 3, HC // 2, S)   # hc 6-7
        load_consts_early()
        load_w("v", 0, HC)
        nc.sync.dma_start(out=bias_row["v"],
                           in_=b_in["v"].ap().unsqueeze(0).broadcast_to([P, HG]))
        load_w("kq", 0, HC)
        load_w("kk", 0, HC)
        load_rows(ehsT_sb, ehsT, 0, HC, TKK)
        load_rows(ehsT_sb, ehsT, 1, HC, TKK)
        load_w("kv", 0, HC)
        load_consts_late()
        # the remaining wq/wk column blocks (jt1-3)
        def load_w_jt13(nm):
            nc.sync.dma_start(
                out=wsb[nm][:, :, P:4 * P],
                in_=w_in[nm][:, P:4 * P].rearrange("(hc p) j -> p hc j", p=P))

        load_w_jt13("q")
        load_w_jt13("k")
        nc.sync.dma_start(out=bias_row["kv"],
                           in_=b_in["kv"].ap().unsqueeze(0).broadcast_to([P, HG]))

        # ---- projection emitters ----
        def proj_t_unit(nm, dst, srcT, jt, sc, nsc):
            """One (jt, sc) unit: 8 hc-chunk matmuls + bias-add."""
            w = 512 if nsc > 1 else TKK
            ps = psproj.tile([P, w], F32, name="psj", tag="psj")
            for hc in range(HC):
                nc.tensor.matmul(
                    ps, lhsT=wsb[nm][:, hc, jt * P:(jt + 1) * P],
                    rhs=srcT[:, hc, sc * w:(sc + 1) * w],
                    start=(hc == 0), stop=(hc == HC - 1))
            nc.vector.tensor_scalar_add(
                dst[:, jt, sc * w:(sc + 1) * w], ps,
                bias_col[nm][:, jt:jt + 1])

        def proj_v_part(nm, dst, srcT, tt, h0, nh):
            # V-projection for a head subrange: each attention branch reads
            # only its own head's Vaug column, so the head-6/7 parts carry
            # end-of-schedule deadlines and can fill the tail gaps
            ps = psproj.tile([P, nh * HD], F32, name=f"psv{tt}_{h0}",
                             tag="psj")
            for hc in range(HC):
                nc.tensor.matmul(
                    ps, lhsT=srcT[:, hc, tt * P:(tt + 1) * P],
                    rhs=wsb[nm][:, hc, h0 * HD:(h0 + nh) * HD],
                    start=(hc == 0), stop=(hc == HC - 1))
            nc.vector.scalar_tensor_tensor(
                out=dst[:, tt, h0:h0 + nh, 0:HD],
                in0=ps.rearrange("p (h d) -> p h d", h=nh),
                scalar=1.0,
                in1=bias_row[nm].rearrange(
                    "p (h d) -> p h d", h=NHL)[:, h0:h0 + nh, :],
                op0=ALU.mult, op1=ALU.add)
            nc.vector.tensor_copy(
                dst[:, tt, h0:h0 + nh, HD:HD + 1],
                twos.unsqueeze(1).broadcast_to([P, nh, 1]))

        # ---- prelude: Q/K jt0 with all four (proj, sc) accumulation groups
        # concurrent (two in the idle scores-psum slots), matmuls woven in
        # DMA-arrival order so the PE starts and p-state-ramps early ----
        pre_ps = {
            ("q", 0): psproj.tile([P, 512], F32, name="pre_q0", tag="psj"),
            ("k", 0): psproj.tile([P, 512], F32, name="pre_k0", tag="psj"),
            ("q", 1): psctx.tile([P, 512], F32, name="pre_q1", tag="ctxps"),
            ("k", 1): psctx.tile([P, 512], F32, name="pre_k1", tag="ctxps"),
        }
        for hc2 in range(HC // 2):
            for nm in ["q", "k"]:
                off = 0 if nm == "q" else P
                for hc in (2 * hc2, 2 * hc2 + 1):
                    for sc in range(2):
                        nc.tensor.matmul(
                            pre_ps[(nm, sc)],
                            lhsT=wqk0_sb[:, hc, off:off + P],
                            rhs=hsT_sb[:, hc, sc * 512:(sc + 1) * 512],
                            start=(hc == 0), stop=(hc == HC - 1))
        # bias-adds ordered so the first scores matmul's inputs (QT s-half 0
        # and KT key-half 0) complete first
        for nm, sc in [("q", 0), ("k", 0), ("q", 1), ("k", 1)]:
            dst = QT if nm == "q" else KT
            nc.vector.tensor_scalar_add(
                dst[:, 0, sc * 512:(sc + 1) * 512], pre_ps[(nm, sc)],
                bias_col[nm][:, 0:1])

        # ---- filler units: remaining projections, drained between branches
        def fill_unit(u):
            nm, jt, sc = u[0], u[1], u[2]
            if nm == "v":
                proj_v_part("v", Vaug, hsT_sb, u[1], u[2], u[3])
            elif nm == "kv":
                proj_v_part("kv", KVaug, ehsT_sb, u[1], u[2], u[3])
            elif nm == "kk":
                proj_t_unit("kk", KKT, ehsT_sb, jt, 0, 1)
            elif nm == "q":
                proj_t_unit("q", QT, hsT_sb, jt, sc, 2)
            elif nm == "k":
                proj_t_unit("k", KT, hsT_sb, jt, sc, 2)
            elif nm == "kq":
                proj_t_unit("kq", KQT, hsT_sb, jt, sc, 2)

        # gap -> units (after self(h) -> G(2h), after knl(h) -> G(2h+1)).
        # Units are woven INSIDE the following branch's kt loop (paced across
        # the kt steps) so ready filler work sits between the ACT-dependent
        # scores/ctx matmuls in the PE queue; leftovers drain right after the
        # branch, which still meets every deadline.
        GAPS = [
            # G0 (inside self0): V heads 0-5, then knl0's projections
            [("v", t, 0, 6) for t in range(8)]
            + [("kq", 0, 0), ("kq", 0, 1), ("kk", 0, 0)],
            # G1 (inside knl0): KV heads 0-5 for knl0's ctx flush
            [("kv", t, 0, 6) for t in range(4)],
            [("q", 1, 0), ("q", 1, 1)],                    # G2
            [("k", 1, 0), ("k", 1, 1)],                    # G3 (self2 needs jt1)
            [("kq", 1, 0), ("kq", 1, 1), ("kk", 1, 0)],    # G4 (knl2 needs jt1)
            [],                                            # G5
            [("q", 2, 0), ("q", 2, 1)],                    # G6
            [("k", 2, 0), ("k", 2, 1)],                    # G7 (self4 needs jt2)
            [("kq", 2, 0), ("kq", 2, 1), ("kk", 2, 0)],    # G8 (knl4 needs jt2)
            [],                                            # G9
            [("q", 3, 0), ("q", 3, 1)],                    # G10
            [("k", 3, 0), ("k", 3, 1)],                    # G11 (self6 needs jt3)
            # G12 (inside self6): kq jt3 + V head 6 (read by sflush(6))
            [("kq", 3, 0), ("kq", 3, 1)] + [("v", t, 6, 1) for t in range(8)],
            # G13 (inside knl6): KV head 6 (read by kflush(6) inside knl7)
            [("kv", t, 6, 1) for t in range(4)],
            # G14 (inside self7): kk jt3 (knl6 follows) + V head 7
            [("kk", 3, 0)] + [("v", t, 7, 1) for t in range(8)],
            # G15 (inside knl7): KV head 7 (read by the final kflush(7))
            [("kv", t, 7, 1) for t in range(4)],
        ]

        # ---- attention branches ----
        # `weave` is a list of callables (filler units, previous-branch ctx
        # flush parts, normalize/merge closures) emitted spread across the kt
        # steps so the PE always has ready work queued between ACT-dependent
        # matmuls.
        def head_branch(h, kt_mat, q_mat, vaug, n_keys, msk, weave,
                        split_last=False):
            base = (h % 2) * HD
            jt = h // 2
            nkt = n_keys // P
            ctxA = psctx.tile([P, 4, HD + 1], F32, name=f"cA_{h}_{n_keys}",
                              tag="ctxps")
            ctxB = psctx.tile([P, 4, HD + 1], F32, name=f"cB_{h}_{n_keys}",
                              tag="ctxps")

            nw = len(weave)
            slots = [min(nkt - 1, ((j + 1) * nkt) // (nw + 1)) for j in range(nw)]
            e_ts = []
            for kt in range(nkt):
                st_ps = psbig.tile([P, S], F32, name=f"st_{h}_{kt}", tag="big")
                for sc2 in range(S // 512):
                    nc.tensor.matmul(
                        st_ps[:, sc2 * 512:(sc2 + 1) * 512],
                        lhsT=kt_mat[base:base + HD, jt, kt * P:(kt + 1) * P],
                        rhs=q_mat[base:base + HD, jt, sc2 * 512:(sc2 + 1) * 512],
                        start=True, stop=True)
                e_t = epool.tile([P, S], BF16, name=f"e_{h}_{kt}", tag="e")
                if split_last and kt == nkt - 1:
                    # halve the last exp so the s<512 ctx groups (and the
                    # final normalize/merge/DMA chain) start half an exp early
                    for eh in range(2):
                        nc.scalar.activation(
                            e_t[:, eh * 512:(eh + 1) * 512],
                            st_ps[:, eh * 512:(eh + 1) * 512], AF.Exp,
                            bias=msk[:, kt:kt + 1], scale=INV)
                else:
                    nc.scalar.activation(e_t, st_ps, AF.Exp,
                                         bias=msk[:, kt:kt + 1], scale=INV)
                if _DUMP and h == 0 and n_keys == TKS:
                    d = nc.dram_tensor(f"d_e0s_{kt}", [P, S], F32,
                                       kind="ExternalOutput")
                    nc.gpsimd.dma_start(out=d.ap(), in_=e_t)
                e_ts.append(e_t)
                for j in range(nw):
                    if slots[j] == kt:
                        weave[j]()

            def flush_part(scs):
                # sc-outer / kt-inner: one open accumulation group per PSUM
                # region at a time (interleaved groups corrupt each other)
                for sc in scs:
                    t = ctxA if sc < 4 else ctxB
                    for kt in range(nkt):
                        nc.tensor.matmul(
                            t[:, sc % 4, :],
                            lhsT=e_ts[kt][:, sc * P:(sc + 1) * P],
                            rhs=vaug[:, kt, h, :],
                            start=(kt == 0), stop=(kt == nkt - 1))

            flush_parts = [lambda scs=(sc0, sc0 + 1): flush_part(scs)
                           for sc0 in range(0, S // P, 2)]
            return (ctxA, ctxB), flush_parts

        def self_branch(h, weave):
            return head_branch(h, KT, QT, Vaug, TKS, mask_sb, weave)

        def knl_branch(h, weave, split_last=False):
            return head_branch(h, KKT, KQT, KVaug, TKK, emask_sb, weave,
                               split_last)

        def norm_part(h, t, i, dst):
            # dst[:, 4i:4i+4, :] = ctx-tile / (2*denominator)
            rb = smallp.tile([P, 4, 1], F32, name=f"rb_{h}_{i}", tag="rb",
                             bufs=4)
            nc.vector.reciprocal(rb, t[:, :, HD:HD + 1])
            nc.vector.tensor_tensor(
                out=dst[:, i * 4:i * 4 + 4, :], in0=t[:, :, 0:HD],
                in1=rb.broadcast_to([P, 4, HD]), op=ALU.mult)

        def out_dma(q, half):
            # head-pair quarter q: heads 2q, 2q+1 -> out columns [128q, 128q+128),
            # s-half `half` -> rows [512*half, 512*half+512)
            oh = out_half[q // 2]
            hp = (2 * q) % 4
            nc.sync.dma_start(
                out=out.ap()[half * 512:(half + 1) * 512,
                             q * P:(q + 1) * P].rearrange(
                    "(sc p) j -> p sc j", p=P),
                in_=oh[:, half * 4:(half + 1) * 4, hp:hp + 2, :].rearrange(
                    "p sc h d -> p sc (h d)"))

        # ---- main schedule: self(h) then knl(h); each branch weaves in the
        # previous branch's ctx flush + normalize/merge plus this gap's
        # projection units, so the ACT-bound exp chain is always overlapped
        # with ready PE work ----
        def units(g):
            return [lambda u=u: fill_unit(u) for u in GAPS[g]]

        # the last four branches run self-self-knl-knl: the knowledge
        # branches have a smaller exp-vs-PE deficit, so putting them at the
        # end (where no projection filler remains) shrinks the tail idle
        schedule = []
        for h in range(6):
            schedule += [("s", h), ("k", h)]
        schedule += [("s", 6), ("s", 7), ("k", 6), ("k", 7)]

        carry = []   # work woven into the next branch
        sN = {}
        for kind, h in schedule:
            if kind == "s":
                ctx_s, sfl = self_branch(h, carry + units(2 * h))

                # flush parts interleaved with the normalize halves they feed
                def mk_sn(i, h=h, ctx_s=ctx_s):
                    def f():
                        if i == 0:
                            sN[h] = snpool.tile([P, S // P, HD], F32,
                                                name=f"sN_{h}", tag="sN")
                        norm_part(h, ctx_s[i], i, sN[h])
                    return f

                carry = [sfl[0], sfl[1], mk_sn(0), sfl[2], sfl[3], mk_sn(1)]
            else:
                ctx_k, kfl = knl_branch(h, carry + units(2 * h + 1),
                                        split_last=(h == 7))

                tmp = {}

                def mk_k(i, h=h, ctx_k=ctx_k, tmp=tmp):
                    def f():
                        if i == 0:
                            tmp[0] = smallp.tile([P, S // P, HD], F32,
                                                 name=f"tK_{h}", tag="tK")
                        norm_part(h + 8, ctx_k[i], i, tmp[0])
                        oh = out_half[h // 4]
                        nc.vector.tensor_tensor(
                            out=oh[:, i * 4:i * 4 + 4, h % 4, :],
                            in0=tmp[0][:, i * 4:i * 4 + 4, :],
                            in1=sN[h][:, i * 4:i * 4 + 4, :], op=ALU.add)
                        if i == 1:
                            sN.pop(h)
                    return f

                carry = [kfl[0], kfl[1], mk_k(0), kfl[2], kfl[3], mk_k(1)]
                if h % 2 == 1:
                    carry.insert(3, lambda q=h // 2: out_dma(q, 0))
                    carry.append(lambda q=h // 2: out_dma(q, 1))
        for f in carry:
            f()

    nc.finalize()
    return nc


def _get_nc():
    if "nc" not in _CACHE:
        _CACHE["nc"] = _build()
    return _CACHE["nc"]


def kernel(**inputs):
    inp = {k: np.asarray(v, dtype=np.float32) for k, v in inputs.items()}
    nc = _get_nc()

    def bf16(x):
        return np.ascontiguousarray(x.astype(ml_dtypes.bfloat16))

    B = 4
    in_maps = []
    for core in range(8):
        b, hg = core // 2, core % 2
        sl = slice(hg * HG, (hg + 1) * HG)
        m = {
            "hsT": bf16(inp["hidden_states"][b].T),
            "ehsT": bf16(inp["encoder_hidden_states"][b].T),
            "wqk0": bf16(np.concatenate(
                [inp["Wq"][:, sl][:, 0:P], inp["Wk"][:, sl][:, 0:P]], axis=1)),
            "mask": np.ascontiguousarray(inp["attention_mask"][b, 0, 0, :]),
            "emask": np.ascontiguousarray(inp["encoder_attention_mask"][b, 0, 0, :]),
        }
        for nm in ["q", "k", "v", "kq", "kk", "kv"]:
            m[f"w{nm}"] = bf16(inp[f"W{nm}"][:, sl])
            m[f"b{nm}"] = np.ascontiguousarray(inp[f"b{nm}"][sl])
        in_maps.append(m)

    res = run_bass_kernel_spmd(nc, in_maps, core_ids=list(range(8)))

    outp = np.empty((B, S, H), np.float32)
    for core in range(8):
        b, hg = core // 2, core % 2
        outp[b, :, hg * HG:(hg + 1) * HG] = res.results[core]["out"]
    return outp


# revision 80
# speedup vs baseline: 1.0152x; 1.0039x over previous
"""Trainium2 Bass kernel for nn_BertSelfAttention_7962869367489.

Dual-branch (self + cross/"knowledge") BERT attention, B=4, S=1024, K=512,
H=1024, NH=16, HD=64, fp32.

Sharding: 8 cores = (batch b in 0..3) x (head-group hg in 0..1, 8 heads each).
All six projections are column-split by head-group; per-head attention is
entirely core-local; output columns are disjoint per core, so the gather is a
pure concatenation (no collectives).

Per-core pipeline (bf16 operands everywhere on the PE; f32 PSUM accumulation):
  - hs/ehs arrive pre-transposed and pre-cast to bf16 from the host
    (hsT [H,S], ehsT [H,K]), weights pre-cast to bf16, so no on-device
    transposes or casts are needed and input DMA bytes are halved.
  - Projections: QT/KT/KQT/KKT = W.T @ srcT in transposed orientation (bf16
    outs); Vaug/KVaug = srcT.T @ Wv in normal orientation with an augmented
    column of 2.0, so the ctx matmul also produces 2*softmax-denominator,
    folding the (ctx+kctx)*0.5 branch average into the normalization.
  - Per head h: scoresT[t,s] = K_h @ Q_h^T (contraction HD=64); exp on ACT
    with per-partition mask bias and 1/8 scale, written bf16; after the
    branch's exps, ctx[s,d|den] accumulates in PSUM in NORMAL orientation
    via lhsT = e-chunk [t,128s], rhs = Vaug_h [t,65] -- 65-row bf16 matmuls,
    ~2.4x fewer PE rows than the transposed form and no output transposes.
    Accumulation groups run sc-outer/kt-inner so each PSUM region hosts one
    group at a time (interleaved groups in one bank clobber each other).
  - Normalization + branch merge on DVE straight out of PSUM; output DMA'd
    in four head-pair quarters.
  - Remaining projections are split into ~1.7us (jt, sc) units and drained
    between attention branches so PE projection work fills the ACT-bound
    exp windows; knowledge branch h runs right after self branch h.
"""
import numpy as np
import ml_dtypes
from contextlib import ExitStack

import concourse.bacc as bacc
import concourse.tile as tile
import concourse.mybir as mybir
from concourse.bass_utils import run_bass_kernel_spmd

F32 = mybir.dt.float32
BF16 = mybir.dt.bfloat16
AF = mybir.ActivationFunctionType
ALU = mybir.AluOpType

P = 128
S = 1024        # query length
TKS = 1024      # self-branch key length
TKK = 512       # knowledge-branch key length
H = 1024        # model dim (projection contraction)
HG = 512        # per-core output width (8 heads x 64)
NHL = 8         # heads per core
HD = 64
HC = H // P     # 8 contraction chunks
INV = 0.125     # 1/sqrt(64)

_CACHE = {}
_DUMP = False


def _build():
    nc = bacc.Bacc(target_bir_lowering=False, debug=False)

    hsT = nc.dram_tensor("hsT", [H, S], BF16, kind="ExternalInput")
    ehsT = nc.dram_tensor("ehsT", [H, TKK], BF16, kind="ExternalInput")
    # host-packed [wq jt0-cols | wk jt0-cols]: contiguous 512B rows dodge the
    # 256B-run DMA penalty on the startup-critical first weight loads
    wqk0 = nc.dram_tensor("wqk0", [H, 2 * P], BF16, kind="ExternalInput")
    w_in = {}
    b_in = {}
    for nm in ["q", "k", "v", "kq", "kk", "kv"]:
        w_in[nm] = nc.dram_tensor(f"w{nm}", [H, HG], BF16, kind="ExternalInput")
        b_in[nm] = nc.dram_tensor(f"b{nm}", [HG], F32, kind="ExternalInput")
    mask = nc.dram_tensor("mask", [TKS], F32, kind="ExternalInput")
    emask = nc.dram_tensor("emask", [TKK], F32, kind="ExternalInput")
    out = nc.dram_tensor("out", [S, HG], F32, kind="ExternalOutput")

    with tile.TileContext(nc) as tc, ExitStack() as ctx:
        const = ctx.enter_context(tc.tile_pool(name="const", bufs=1))
        persist = ctx.enter_context(tc.tile_pool(name="persist", bufs=1))
        epool = ctx.enter_context(tc.tile_pool(name="epool", bufs=17))
        smallp = ctx.enter_context(tc.tile_pool(name="smallp", bufs=2))
        snpool = ctx.enter_context(tc.tile_pool(name="snpool", bufs=3))
        psproj = ctx.enter_context(tc.tile_pool(name="psproj", bufs=2, space="PSUM"))
        psbig = ctx.enter_context(tc.tile_pool(name="psbig", bufs=2, space="PSUM"))
        psctx = ctx.enter_context(tc.tile_pool(name="psctx", bufs=2, space="PSUM"))

        # ---- constants (gpsimd/SWDGE queue, but the DMA engines are shared,
        # so these are emitted interleaved with the big loads below in
        # need-order to keep the startup-critical stream dense) ----
        mask_sb = const.tile([P, TKS // P], F32)
        emask_sb = const.tile([P, TKK // P], F32)
        bias_col = {}
        for nm in ["q", "k", "kq", "kk"]:
            bias_col[nm] = const.tile([P, 4], F32, name=f"bias_{nm}")
        bias_row = {}
        for nm in ["v", "kv"]:
            bias_row[nm] = const.tile([P, HG], F32, name=f"brow_{nm}")
        twos = const.tile([P, 1], F32)
        nc.vector.memset(twos, 2.0)

        def load_consts_early():
            for nm in ["q", "k"]:
                nc.sync.dma_start(
                    out=bias_col[nm],
                    in_=b_in[nm].ap().rearrange("(jt p) -> p jt", p=P))
            nc.sync.dma_start(out=mask_sb,
                              in_=mask.ap().rearrange("(kt p) -> p kt", p=P))

        def load_consts_late():
            nc.sync.dma_start(out=emask_sb,
                              in_=emask.ap().rearrange("(kt p) -> p kt", p=P))
            for nm in ["kq", "kk"]:
                nc.sync.dma_start(
                    out=bias_col[nm],
                    in_=b_in[nm].ap().rearrange("(jt p) -> p jt", p=P))

        # ---- persistent activations ----
        QT = persist.tile([P, 4, S], BF16)       # [j%128, jt, s]
        KT = persist.tile([P, 4, TKS], BF16)
        KQT = persist.tile([P, 4, S], BF16)
        KKT = persist.tile([P, 4, TKK], BF16)
        Vaug = persist.tile([P, TKS // P, NHL, HD + 1], BF16)   # [t%128, tt, h, d|2]
        KVaug = persist.tile([P, TKK // P, NHL, HD + 1], BF16)
        hsT_sb = persist.tile([P, HC, S], BF16)   # [h%128, hc, s]
        ehsT_sb = persist.tile([P, HC, TKK], BF16)
        wsb = {}
        for nm in ["q", "k", "v", "kq", "kk", "kv"]:
            wsb[nm] = persist.tile([P, HC, HG], BF16, name=f"w_{nm}")
        wqk0_sb = persist.tile([P, HC, 2 * P], BF16)
        # output staging in two head-halves; DMA'd in four head-pair quarters
        out_half = [persist.tile([P, S // P, 4, HD], F32, name=f"out_half{i}",
                                 tag=f"out_half{i}") for i in range(2)]

        # ---- input DMAs (sync/HWDGE queue), ordered so the prelude's
        # dependencies (hsT, wq, wk) land first ----
        def load_rows(dst, src, half, rows, cols):
            nc.sync.dma_start(
                out=dst[:, half * (rows // 2):(half + 1) * (rows // 2), :],
                in_=src[half * (rows * P // 2):(half + 1) * (rows * P // 2), :]
                .rearrange("(hc p) s -> p hc s", p=P))

        def load_w(nm, hc0, hcn):
            nc.sync.dma_start(
                out=wsb[nm][:, hc0:hc0 + hcn, :],
                in_=w_in[nm][hc0 * P:(hc0 + hcn) * P, :].rearrange(
                    "(hc p) j -> p hc j", p=P))

        def load_wqk0(hc0, hcn):
            nc.sync.dma_start(
                out=wqk0_sb[:, hc0:hc0 + hcn, :],
                in_=wqk0[hc0 * P:(hc0 + hcn) * P, :].rearrange(
                    "(hc p) j -> p hc j", p=P))

        # startup-critical loads (prelude needs wq/wk jt0 + all of hsT),
        # split fine and interleaved so the first projection matmuls start
        # (and the PE p-state ramps) as early as possible
        load_wqk0(0, 2)
        nc.sync.dma_start(out=hsT_sb[:, 0:1, :], in_=hsT[0:P, :].rearrange(
            "(hc p) s -> p hc s", p=P))
        load_wqk0(2, 2)
        nc.sync.dma_start(out=hsT_sb[:, 1:2, :], in_=hsT[P:2 * P, :].rearrange(
            "(hc p) s -> p hc s", p=P))
        load_wqk0(4, 2)
        load_rows(hsT_sb, hsT, 1, HC // 2, S)   # hc 2-3
        load_wqk0(6, 2)
        load_rows(hsT_sb, hsT, 2, HC // 2, S)   # hc 4-5
        load_rows(hsT_sb, hsT, 3, HC // 2, S)   # hc 6-7
        load_consts_early()
        load_w("v", 0, HC)
        nc.sync.dma_start(out=bias_row["v"],
                           in_=b_in["v"].ap().unsqueeze(0).broadcast_to([P, HG]))
        load_w("kq", 0, HC)
        load_w("kk", 0, HC)
        load_rows(ehsT_sb, ehsT, 0, HC, TKK)
        load_rows(ehsT_sb, ehsT, 1, HC, TKK)
        load_w("kv", 0, HC)
        load_consts_late()
        # the remaining wq/wk column blocks (jt1-3)
        def load_w_jt13(nm):
            nc.sync.dma_start(
                out=wsb[nm][:, :, P:4 * P],
                in_=w_in[nm][:, P:4 * P].rearrange("(hc p) j -> p hc j", p=P))

        load_w_jt13("q")
        load_w_jt13("k")
        nc.sync.dma_start(out=bias_row["kv"],
                           in_=b_in["kv"].ap().unsqueeze(0).broadcast_to([P, HG]))

        # ---- projection emitters ----
        def proj_t_unit(nm, dst, srcT, jt, sc, nsc):
            """One (jt, sc) unit: 8 hc-chunk matmuls + bias-add."""
            w = 512 if nsc > 1 else TKK
            ps = psproj.tile([P, w], F32, name="psj", tag="psj")
            for hc in range(HC):
                nc.tensor.matmul(
                    ps, lhsT=wsb[nm][:, hc, jt * P:(jt + 1) * P],
                    rhs=srcT[:, hc, sc * w:(sc + 1) * w],
                    start=(hc == 0), stop=(hc == HC - 1))
            nc.vector.tensor_scalar_add(
                dst[:, jt, sc * w:(sc + 1) * w], ps,
                bias_col[nm][:, jt:jt + 1])

        def proj_v_part(nm, dst, srcT, tt, h0, nh):
            # V-projection for a head subrange: each attention branch reads
            # only its own head's Vaug column, so the head-6/7 parts carry
            # end-of-schedule deadlines and can fill the tail gaps
            ps = psproj.tile([P, nh * HD], F32, name=f"psv{tt}_{h0}",
                             tag="psj")
            for hc in range(HC):
                nc.tensor.matmul(
                    ps, lhsT=srcT[:, hc, tt * P:(tt + 1) * P],
                    rhs=wsb[nm][:, hc, h0 * HD:(h0 + nh) * HD],
                    start=(hc == 0), stop=(hc == HC - 1))
            nc.vector.scalar_tensor_tensor(
                out=dst[:, tt, h0:h0 + nh, 0:HD],
                in0=ps.rearrange("p (h d) -> p h d", h=nh),
                scalar=1.0,
                in1=bias_row[nm].rearrange(
                    "p (h d) -> p h d", h=NHL)[:, h0:h0 + nh, :],
                op0=ALU.mult, op1=ALU.add)
            nc.vector.tensor_copy(
                dst[:, tt, h0:h0 + nh, HD:HD + 1],
                twos.unsqueeze(1).broadcast_to([P, nh, 1]))

        # ---- prelude: Q/K jt0 with all four (proj, sc) accumulation groups
        # concurrent (two in the idle scores-psum slots), matmuls woven in
        # DMA-arrival order so the PE starts and p-state-ramps early ----
        pre_ps = {
            ("q", 0): psproj.tile([P, 512], F32, name="pre_q0", tag="psj"),
            ("k", 0): psproj.tile([P, 512], F32, name="pre_k0", tag="psj"),
            ("q", 1): psctx.tile([P, 512], F32, name="pre_q1", tag="ctxps"),
            ("k", 1): psctx.tile([P, 512], F32, name="pre_k1", tag="ctxps"),
        }
        for hc2 in range(HC // 2):
            for nm in ["q", "k"]:
                off = 0 if nm == "q" else P
                for hc in (2 * hc2, 2 * hc2 + 1):
                    for sc in range(2):
                        nc.tensor.matmul(
                            pre_ps[(nm, sc)],
                            lhsT=wqk0_sb[:, hc, off:off + P],
                            rhs=hsT_sb[:, hc, sc * 512:(sc + 1) * 512],
                            start=(hc == 0), stop=(hc == HC - 1))
        # bias-adds ordered so the first scores matmul's inputs (QT s-half 0
        # and KT key-half 0) complete first
        for nm, sc in [("q", 0), ("k", 0), ("q", 1), ("k", 1)]:
            dst = QT if nm == "q" else KT
            nc.vector.tensor_scalar_add(
                dst[:, 0, sc * 512:(sc + 1) * 512], pre_ps[(nm, sc)],
                bias_col[nm][:, 0:1])

        # ---- filler units: remaining projections, drained between branches
        def fill_unit(u):
            nm, jt, sc = u[0], u[1], u[2]
            if nm == "v":
                proj_v_part("v", Vaug, hsT_sb, u[1], u[2], u[3])
            elif nm == "kv":
                proj_v_part("kv", KVaug, ehsT_sb, u[1], u[2], u[3])
            elif nm == "kk":
                proj_t_unit("kk", KKT, ehsT_sb, jt, 0, 1)
            elif nm == "q":
                proj_t_unit("q", QT, hsT_sb, jt, sc, 2)
            elif nm == "k":
                proj_t_unit("k", KT, hsT_sb, jt, sc, 2)
            elif nm == "kq":
                proj_t_unit("kq", KQT, hsT_sb, jt, sc, 2)

        # gap -> units (after self(h) -> G(2h), after knl(h) -> G(2h+1)).
        # Units are woven INSIDE the following branch's kt loop (paced across
        # the kt steps) so ready filler work sits between the ACT-dependent
        # scores/ctx matmuls in the PE queue; leftovers drain right after the
        # branch, which still meets every deadline.
        GAPS = [
            # G0 (inside self0): V heads 0-5, then knl0's projections
            [("v", t, 0, 6) for t in range(8)]
            + [("kq", 0, 0), ("kq", 0, 1), ("kk", 0, 0)],
            # G1 (inside knl0): KV heads 0-5 for knl0's ctx flush
            [("kv", t, 0, 6) for t in range(4)],
            [("q", 1, 0), ("q", 1, 1)],                    # G2
            [("k", 1, 0), ("k", 1, 1)],                    # G3 (self2 needs jt1)
            [("kq", 1, 0), ("kq", 1, 1), ("kk", 1, 0)],    # G4 (knl2 needs jt1)
            [],                                            # G5
            [("q", 2, 0), ("q", 2, 1)],                    # G6
            [("k", 2, 0), ("k", 2, 1)],                    # G7 (self4 needs jt2)
            [("kq", 2, 0), ("kq", 2, 1), ("kk", 2, 0)],    # G8 (knl4 needs jt2)
            [],                                            # G9
            [("q", 3, 0), ("q", 3, 1)],                    # G10
            [("k", 3, 0), ("k", 3, 1)],                    # G11 (self6 needs jt3)
            # G12 (inside self6): kq jt3 + V head 6 (read by sflush(6))
            [("kq", 3, 0), ("kq", 3, 1)] + [("v", t, 6, 1) for t in range(8)],
            # G13 (inside knl6): KV head 6 (read by kflush(6) inside knl7)
            [("kv", t, 6, 1) for t in range(4)],
            # G14 (inside self7): kk jt3 (knl6 follows) + V head 7
            [("kk", 3, 0)] + [("v", t, 7, 1) for t in range(8)],
            # G15 (inside knl7): KV head 7 (read by the final kflush(7))
            [("kv", t, 7, 1) for t in range(4)],
        ]

        # ---- attention branches ----
        # `weave` is a list of callables (filler units, previous-branch ctx
        # flush parts, normalize/merge closures) emitted spread across the kt
        # steps so the PE always has ready work queued between ACT-dependent
        # matmuls.
        def head_branch(h, kt_mat, q_mat, vaug, n_keys, msk, weave,
                        split_last=False):
            base = (h % 2) * HD
            jt = h // 2
            nkt = n_keys // P
            ctxA = psctx.tile([P, 4, HD + 1], F32, name=f"cA_{h}_{n_keys}",
                              tag="ctxps")
            ctxB = psctx.tile([P, 4, HD + 1], F32, name=f"cB_{h}_{n_keys}",
                              tag="ctxps")

            nw = len(weave)
            slots = [min(nkt - 1, ((j + 1) * nkt) // (nw + 1)) for j in range(nw)]
            e_ts = []
            for kt in range(nkt):
                st_ps = psbig.tile([P, S], F32, name=f"st_{h}_{kt}", tag="big")
                for sc2 in range(S // 512):
                    nc.tensor.matmul(
                        st_ps[:, sc2 * 512:(sc2 + 1) * 512],
                        lhsT=kt_mat[base:base + HD, jt, kt * P:(kt + 1) * P],
                        rhs=q_mat[base:base + HD, jt, sc2 * 512:(sc2 + 1) * 512],
                        start=True, stop=True)
                e_t = epool.tile([P, S], BF16, name=f"e_{h}_{kt}", tag="e")
                if split_last and kt >= nkt - 2:
                    # halve the last exp so the s<512 ctx groups (and the
                    # final normalize/merge/DMA chain) start half an exp early
                    for eh in range(2):
                        nc.scalar.activation(
                            e_t[:, eh * 512:(eh + 1) * 512],
                            st_ps[:, eh * 512:(eh + 1) * 512], AF.Exp,
                            bias=msk[:, kt:kt + 1], scale=INV)
                else:
                    nc.scalar.activation(e_t, st_ps, AF.Exp,
                                         bias=msk[:, kt:kt + 1], scale=INV)
                if _DUMP and h == 0 and n_keys == TKS:
                    d = nc.dram_tensor(f"d_e0s_{kt}", [P, S], F32,
                                       kind="ExternalOutput")
                    nc.gpsimd.dma_start(out=d.ap(), in_=e_t)
                e_ts.append(e_t)
                for j in range(nw):
                    if slots[j] == kt:
                        weave[j]()

            def flush_part(scs):
                # sc-outer / kt-inner: one open accumulation group per PSUM
                # region at a time (interleaved groups corrupt each other)
                for sc in scs:
                    t = ctxA if sc < 4 else ctxB
                    for kt in range(nkt):
                        nc.tensor.matmul(
                            t[:, sc % 4, :],
                            lhsT=e_ts[kt][:, sc * P:(sc + 1) * P],
                            rhs=vaug[:, kt, h, :],
                            start=(kt == 0), stop=(kt == nkt - 1))

            flush_parts = [lambda scs=(sc0, sc0 + 1): flush_part(scs)
                           for sc0 in range(0, S // P, 2)]
            return (ctxA, ctxB), flush_parts

        def self_branch(h, weave):
            return head_branch(h, KT, QT, Vaug, TKS, mask_sb, weave)

        def knl_branch(h, weave, split_last=False):
            return head_branch(h, KKT, KQT, KVaug, TKK, emask_sb, weave,
                               split_last)

        def norm_part(h, t, i, dst):
            # dst[:, 4i:4i+4, :] = ctx-tile / (2*denominator)
            rb = smallp.tile([P, 4, 1], F32, name=f"rb_{h}_{i}", tag="rb",
                             bufs=4)
            nc.vector.reciprocal(rb, t[:, :, HD:HD + 1])
            nc.vector.tensor_tensor(
                out=dst[:, i * 4:i * 4 + 4, :], in0=t[:, :, 0:HD],
                in1=rb.broadcast_to([P, 4, HD]), op=ALU.mult)

        def out_dma(q, half):
            # head-pair quarter q: heads 2q, 2q+1 -> out columns [128q, 128q+128),
            # s-half `half` -> rows [512*half, 512*half+512)
            oh = out_half[q // 2]
            hp = (2 * q) % 4
            nc.sync.dma_start(
                out=out.ap()[half * 512:(half + 1) * 512,
                             q * P:(q + 1) * P].rearrange(
                    "(sc p) j -> p sc j", p=P),
                in_=oh[:, half * 4:(half + 1) * 4, hp:hp + 2, :].rearrange(
                    "p sc h d -> p sc (h d)"))

        # ---- main schedule: self(h) then knl(h); each branch weaves in the
        # previous branch's ctx flush + normalize/merge plus this gap's
        # projection units, so the ACT-bound exp chain is always overlapped
        # with ready PE work ----
        def units(g):
            return [lambda u=u: fill_unit(u) for u in GAPS[g]]

        # the last four branches run self-self-knl-knl: the knowledge
        # branches have a smaller exp-vs-PE deficit, so putting them at the
        # end (where no projection filler remains) shrinks the tail idle
        schedule = []
        for h in range(6):
            schedule += [("s", h), ("k", h)]
        schedule += [("s", 6), ("s", 7), ("k", 6), ("k", 7)]

        carry = []   # work woven into the next branch
        sN = {}
        for kind, h in schedule:
            if kind == "s":
                ctx_s, sfl = self_branch(h, units(2 * h) + carry)

                # flush parts interleaved with the normalize halves they feed
                def mk_sn(i, h=h, ctx_s=ctx_s):
                    def f():
                        if i == 0:
                            sN[h] = snpool.tile([P, S // P, HD], F32,
                                                name=f"sN_{h}", tag="sN")
                        norm_part(h, ctx_s[i], i, sN[h])
                    return f

                carry = [sfl[0], sfl[1], mk_sn(0), sfl[2], sfl[3], mk_sn(1)]
            else:
                ctx_k, kfl = knl_branch(h, units(2 * h + 1) + carry,
                                        split_last=(h == 7))

                tmp = {}

                def mk_k(i, h=h, ctx_k=ctx_k, tmp=tmp):
                    def f():
                        if i == 0:
                            tmp[0] = smallp.tile([P, S // P, HD], F32,
                                                 name=f"tK_{h}", tag="tK")
                        norm_part(h + 8, ctx_k[i], i, tmp[0])
                        oh = out_half[h // 4]
                        nc.vector.tensor_tensor(
                            out=oh[:, i * 4:i * 4 + 4, h % 4, :],
                            in0=tmp[0][:, i * 4:i * 4 + 4, :],
                            in1=sN[h][:, i * 4:i * 4 + 4, :], op=ALU.add)
                        if i == 1:
                            sN.pop(h)
                    return f

                carry = [kfl[0], kfl[1], mk_k(0), kfl[2], kfl[3], mk_k(1)]
                if h % 2 == 1:
                    carry.insert(3, lambda q=h // 2: out_dma(q, 0))
                    carry.append(lambda q=h // 2: out_dma(q, 1))
        for f in carry:
            f()

    nc.finalize()
    return nc


def _get_nc():
    if "nc" not in _CACHE:
        _CACHE["nc"] = _build()
    return _CACHE["nc"]


def kernel(**inputs):
    inp = {k: np.asarray(v, dtype=np.float32) for k, v in inputs.items()}
    nc = _get_nc()

    def bf16(x):
        return np.ascontiguousarray(x.astype(ml_dtypes.bfloat16))

    B = 4
    in_maps = []
    for core in range(8):
        b, hg = core // 2, core % 2
        sl = slice(hg * HG, (hg + 1) * HG)
        m = {
            "hsT": bf16(inp["hidden_states"][b].T),
            "ehsT": bf16(inp["encoder_hidden_states"][b].T),
            "wqk0": bf16(np.concatenate(
                [inp["Wq"][:, sl][:, 0:P], inp["Wk"][:, sl][:, 0:P]], axis=1)),
            "mask": np.ascontiguousarray(inp["attention_mask"][b, 0, 0, :]),
            "emask": np.ascontiguousarray(inp["encoder_attention_mask"][b, 0, 0, :]),
        }
        for nm in ["q", "k", "v", "kq", "kk", "kv"]:
            m[f"w{nm}"] = bf16(inp[f"W{nm}"][:, sl])
            m[f"b{nm}"] = np.ascontiguousarray(inp[f"b{nm}"][sl])
        in_maps.append(m)

    res = run_bass_kernel_spmd(nc, in_maps, core_ids=list(range(8)))

    outp = np.empty((B, S, H), np.float32)
    for core in range(8):
        b, hg = core // 2, core % 2
        outp[b, :, hg * HG:(hg + 1) * HG] = res.results[core]["out"]
    return outp
